# revision 6
# baseline (speedup 1.0000x reference)
"""Trainium2 Bass kernel for nn_InvariantMapping (topk_masking) — v2.

Math: score[b,n] = px.G.py with G_dd' = sum_c fx_d fy_d', px/py the
normalized channel means. Softmax is monotonic, so top-k needs only raw
scores, and the final gather output is exact fp32 values from the host
arrays. Only the RANKING comes from the device, so the device may compute
an approximate score as long as the true top-8 survives inside a candidate
set that the host then re-scores exactly in fp64.

v3 pipeline (modeled 0.445 ms/core via TimelineSim == the PE floor: the
tensor engine is 100% busy streaming 30 matmuls x 512 cols per 512-pt
chunk; DVE products and ACT casts are fully hidden underneath):
 - host quantizes fx/fy to int8 (sym scale) -> 4x less tunnel + HBM traffic
 - device: DMA int8 [128c,3,2048] tiles, ACT-casts to fp16 at 1024-wide
   granularity, DVE forms the 9 Gram products per c-group (broadcast over
   d'), PE reduces over channels with a ones[128,32] fp16 stationary at
   1 cycle/row (fp32 PSUM, group0 start / group1 stop), 4 comps per PSUM
   bank at partition bases {0,32,64,96}, 4 banks double-buffered = all 8
   banks; PSUM evicted to SBUF split across ACT/DVE (Pool cannot touch
   PSUM - BIR verifier rejects), then DMA'd out
 - host: approx score from the 15 sums, top-512 candidates per batch,
   exact fp64 rescore of candidates from the original fp32 arrays
   (audited on real HW comps: approx rank of every true top-8 point is
   <= 8, a 64x margin), stable tie-break by index to match jax top_k,
   exact gather

Sharding: data-parallel over batch, 2 batches per core on 8 cores.
"""
import sys

sys.path.insert(0, "/opt/trn_rl_repo")

import numpy as np

B, C, D, NPTS = 16, 256, 3, 16384
NCORES = 8
BPC = B // NCORES
NT_IO = 2048   # int8 DMA tile: 2KB contiguous per (c,d) row
NSUB = 512     # PSUM chunk (one fp32 bank)
NSUBT = NPTS // NSUB
EPS = 1e-6
NCAND = 512    # candidates per batch for the exact host rescore
QMAX = 126.0

_CACHE = {}


def _build_nc(bpc=BPC, npts=NPTS, nt_io=NT_IO, nsub=NSUB):
    import concourse.bacc as bacc
    import concourse.bass as bass
    import concourse.mybir as mybir
    import concourse.tile as tile

    f32 = mybir.dt.float32
    f16 = mybir.dt.float16
    i8 = mybir.dt.int8
    nsubt = npts // nsub
    subs = nt_io // nsub

    nc = bacc.Bacc()
    fxs = nc.dram_tensor("fxs", [bpc, C, D, npts], i8, kind="ExternalInput")
    fys = nc.dram_tensor("fys", [bpc, C, D, npts], i8, kind="ExternalInput")
    comps = nc.dram_tensor("comps", [bpc, nsubt, 16, nsub], f32, kind="ExternalOutput")

    with tile.TileContext(nc) as tc:
        with (
            tc.tile_pool(name="io", bufs=2) as iop,
            tc.tile_pool(name="cast", bufs=2) as castp,
            tc.tile_pool(name="prod", bufs=9) as prodp,
            tc.tile_pool(name="onesp", bufs=1) as onesp,
            tc.tile_pool(name="ps0", bufs=2, space="PSUM") as ps0,
            tc.tile_pool(name="ps1", bufs=2, space="PSUM") as ps1,
            tc.tile_pool(name="ps2", bufs=2, space="PSUM") as ps2,
            tc.tile_pool(name="ps3", bufs=2, space="PSUM") as ps3,
            tc.tile_pool(name="strip", bufs=2) as stripp,
        ):
            psp = [ps0, ps1, ps2, ps3]
            ones16 = onesp.tile([128, 32], f16)
            nc.vector.memset(ones16, 1.0)

            for b in range(bpc):
                for t in range(npts // nt_io):
                    n0 = nt_io * t
                    xi, yi = [], []
                    for g in range(2):
                        c0 = 128 * g
                        xg = iop.tile([128, D, nt_io], i8, tag=f"xi{g}")
                        yg = iop.tile([128, D, nt_io], i8, tag=f"yi{g}")
                        nc.sync.dma_start(
                            out=xg, in_=fxs[b, c0 : c0 + 128, :, n0 : n0 + nt_io]
                        )
                        nc.sync.dma_start(
                            out=yg, in_=fys[b, c0 : c0 + 128, :, n0 : n0 + nt_io]
                        )
                        xi.append(xg)
                        yi.append(yg)

                    for s in range(subs):
                        m0 = nsub * s
                        ts = t * subs + s
                        if s % 2 == 0:
                            xh2, yh2 = [], []
                            for g in range(2):
                                xc = castp.tile(
                                    [128, D, 2 * nsub], f16, tag=f"xh{g}"
                                )
                                yc = castp.tile(
                                    [128, D, 2 * nsub], f16, tag=f"yh{g}"
                                )
                                nc.scalar.copy(
                                    out=xc, in_=xi[g][:, :, m0 : m0 + 2 * nsub]
                                )
                                nc.scalar.copy(
                                    out=yc, in_=yi[g][:, :, m0 : m0 + 2 * nsub]
                                )
                                xh2.append(xc)
                                yh2.append(yc)
                        h0 = (s % 2) * nsub
                        xh = [t2[:, :, h0 : h0 + nsub] for t2 in xh2]
                        yh = [t2[:, :, h0 : h0 + nsub] for t2 in yh2]

                        # 9 Gram products per c-group: pr[g][d][:, dp, :] =
                        # xh_d * yh_dp  (broadcast xh_d over dp)
                        pr = {}
                        for g in range(2):
                            for d in range(D):
                                p = prodp.tile([128, D, nsub], f16, tag=f"pr{g}")
                                nc.vector.tensor_mul(
                                    p,
                                    xh[g][:, d : d + 1, :].to_broadcast(
                                        [128, D, nsub]
                                    ),
                                    yh[g],
                                )
                                pr[(g, d)] = p

                        banks = [
                            psp[j].tile([128, nsub], f32, tag="bk", name=f"bank{j}")
                            for j in range(4)
                        ]
                        for k in range(15):
                            j, r = k // 4, 32 * (k % 4)
                            out = banks[j][r : r + 32, :]
                            for g in range(2):
                                if k < 3:
                                    rhs = xh[g][:, k, :]
                                elif k < 6:
                                    rhs = yh[g][:, k - 3, :]
                                else:
                                    m = k - 6
                                    rhs = pr[(g, m // 3)][:, m % 3, :]
                                nc.tensor.matmul(
                                    out,
                                    ones16,
                                    rhs,
                                    start=(g == 0),
                                    stop=(g == 1),
                                    tile_position=(0, r),
                                )

                        # evict PSUM->SBUF (2 banks on ACT, 2 on DVE), then DMA
                        for j in range(4):
                            nrow = 4 if j < 3 else 3
                            st = stripp.tile(
                                [128, nsub], f32, tag=f"st{j}", name=f"st{j}"
                            )
                            np_used = 32 * nrow
                            if j < 2:
                                nc.scalar.copy(
                                    out=st[:np_used], in_=banks[j][:np_used]
                                )
                            else:
                                nc.vector.tensor_scalar_add(
                                    st[:np_used], banks[j][:np_used], 0.0
                                )
                            strided = bass.AP(
                                tensor=st.tensor,
                                offset=st.offset,
                                ap=[[32 * st.ap[0][0], nrow]] + list(st.ap[1:]),
                            )
                            nc.sync.dma_start(
                                out=comps[b, ts, 4 * j : 4 * j + nrow], in_=strided
                            )
    nc.finalize()
    return nc


def _get_nc():
    if "nc" not in _CACHE:
        _CACHE["nc"] = _build_nc()
    return _CACHE["nc"]


def _quantize(fx, fy):
    # s = 126/amax guarantees |x*s| <= 126, so no clip is needed; chunked
    # in-place passes keep the fp32 temporary cache-resident (5x faster
    # than the naive 4-temporary chain, bit-identical output)
    amax = max(
        float(fx.max()), -float(fx.min()), float(fy.max()), -float(fy.min())
    )
    s = QMAX / max(amax, 1e-30)
    ch = 1 << 22
    tmp = np.empty(ch, np.float32)
    outs = []
    for a in (fx, fy):
        flat = a.reshape(-1)
        q = np.empty(flat.shape, np.int8)
        for i in range(0, flat.size, ch):
            j = min(i + ch, flat.size)
            t = tmp[: j - i]
            np.multiply(flat[i:j], s, out=t)
            np.rint(t, out=t)
            q[i:j] = t
        outs.append(q.reshape(a.shape))
    return outs[0], outs[1]


def _run_device(xi, yi, trace=False):
    from concourse.bass_utils import run_bass_kernel_spmd

    nc = _get_nc()
    in_maps = []
    for i in range(NCORES):
        sl = slice(BPC * i, BPC * (i + 1))
        in_maps.append({"fxs": xi[sl], "fys": yi[sl]})
    res = run_bass_kernel_spmd(nc, in_maps, core_ids=list(range(NCORES)), trace=trace)
    out = np.stack([r["comps"] for r in res.results])  # [8, BPC, NSUBT, 16, NSUB]
    return out, res


def _approx_scores(out):
    # out: [8, BPC, NSUBT, 16, NSUB] -> [B, 16, NPTS]; comp k at row k
    a = np.asarray(out, np.float64)
    a = a.transpose(0, 1, 3, 2, 4).reshape(NCORES * BPC, 16, NPTS)
    Sx = a[:, 0:3]
    Sy = a[:, 3:6]
    G = a[:, 6:15].reshape(B, 3, 3, NPTS)
    nx = np.sqrt((Sx**2).sum(1, keepdims=True)) + EPS
    ny = np.sqrt((Sy**2).sum(1, keepdims=True)) + EPS
    px = Sx / nx
    py = Sy / ny
    return np.einsum("bdn,bden,ben->bn", px, G, py)


def _exact_topk(fx, fy, cand, kk):
    # exact fp64 rescore of candidate columns; returns [B, kk] indices in
    # jax.lax.top_k order (desc value, ties -> lower index)
    idx = np.empty((B, kk), np.int64)
    for b in range(B):
        cols = np.sort(cand[b])
        fxc = fx[b][:, :, cols].astype(np.float64)  # [C, D, m]
        fyc = fy[b][:, :, cols].astype(np.float64)
        mx = fxc.mean(0)  # [D, m]
        my = fyc.mean(0)
        px = mx / (np.sqrt((mx**2).sum(0, keepdims=True)) + EPS)
        py = my / (np.sqrt((my**2).sum(0, keepdims=True)) + EPS)
        phix = np.einsum("cdm,dm->mc", fxc, px)
        phiy = np.einsum("cdm,dm->mc", fyc, py)
        s = np.einsum("mc,mc->m", phix, phiy)
        order = np.argsort(-s, kind="stable")[:kk]
        idx[b] = cols[order]
    return idx


def kernel(fx, fy, topk):
    fx = np.asarray(fx, dtype=np.float32)
    fy = np.asarray(fy, dtype=np.float32)
    kk = B // int(topk)

    xi, yi = _quantize(fx, fy)
    out, _ = _run_device(xi, yi)
    score = _approx_scores(out)

    ncand = max(NCAND, kk)
    cand = np.argpartition(-score, ncand - 1, axis=1)[:, :ncand]
    idx = _exact_topk(fx, fy, cand, kk).astype(np.int32)

    idxe = idx[:, None, None, :]
    fx_sel = np.take_along_axis(fx, idxe, axis=3)
    fy_sel = np.take_along_axis(fy, idxe, axis=3)
    return (fx_sel, fy_sel)
